# revision 1
# baseline (speedup 1.0000x reference)
"""Trainium2 Bass kernel for the AttentionBlock problem.

Math (per batch b):
  x_down = avgpool4x4(x)            # [C, 32, 32] -> xf [C, N], N=1024
  q,k = Wq/Wk @ xf + b              # [8, N]
  v = Wv @ xf + bv                  # [C, N]
  attn = softmax_n(q^T k)           # [N, N]
  out[c,m] = sum_n v[c,n] attn[m,n]
  y = gamma * upsample_bilinear(out) + x

v2: x is staged to the device as bf16 and y is written back as bf16
(halves the DMA traffic; rel err ~2e-3 vs the 2e-2 gate). The residual
add is split between DVE (fused psum+x adds) and ACT (psum->sbuf copy
after an identity-weight matmul folds x into the psum tile) since only
DVE/ACT may touch PSUM. Bilinear row structure (out rows 0-1 read
hb-1/hb, rows 2-3 read hb/hb+1) lets straddling upsample tiles issue a
256-column minor matmul instead of a full 512. attnV for m-chunks 0/1
is pre-accumulated during the input phase so the first output tiles
start right after the last logits exp.

Mapping (one NeuronCore per batch, 8 cores):
  - x resident in SBUF as two [128, 16384] tiles (c-halves); pooled via DVE
    (w-reduce + h pair adds). The 1/16 mean factor is folded into the weights.
  - q,k computed together (lhsT = [WqT|WkT], M=16), f32r matmuls.
  - logits computed transposed: Lt[n, m] = k^T q, n on partitions -> exp on ACT
    -> Et bf16. Vt[n, c] computed directly (lhsT = xf chunk), ones column
    appended for the softmax denominator.
  - attn@V: O[m, c] = Et^T Vt accumulated over n-chunks; denominator lands in
    column 256; normalize via per-partition reciprocal * tensor_scalar.
  - Upsample fused into one sparse matmul: y[c, (H,W)-slice] = sum_m O[m, c] *
    slab[m, slice], slab[m, (H,W)] = gamma*U[H, hb(m)]*U[W, wb(m)] generated
    on-device (4 ACT ops per slab). Residual add fused into the PSUM->SBUF
    copy (DVE tensor_add with x), written back in place over x, then DMA out.
"""

import numpy as np

B, C, H, W = 8, 256, 128, 128
HD, WD = 32, 32
N = HD * WD  # 1024
CQ = 8
NCORES = 8

_CACHE = {}


def _resize_matrix(dst: int, src: int) -> np.ndarray:
    """Bilinear (half-pixel, edge-renormalized) resize matrix, matches
    jax.image.resize(method='linear') for upsampling."""
    scale = dst / src
    pos = (np.arange(dst, dtype=np.float64) + 0.5) / scale - 0.5
    j = np.arange(src, dtype=np.float64)
    w = np.maximum(0.0, 1.0 - np.abs(pos[:, None] - j[None, :]))
    w = w / w.sum(axis=1, keepdims=True)
    return w.astype(np.float32)  # [dst, src]


def _build_bass():
    import concourse.bass as bass
    import concourse.tile as tile
    from concourse import bacc, mybir

    f32 = mybir.dt.float32
    f32r = mybir.dt.float32r
    bf16 = mybir.dt.bfloat16
    AF = mybir.ActivationFunctionType
    AX = mybir.AxisListType
    AL = mybir.AluOpType

    nc = bacc.Bacc("TRN2", target_bir_lowering=False, debug=False)

    x_d = nc.dram_tensor("x", [C, H * W], bf16, kind="ExternalInput")
    wqk_d = nc.dram_tensor("wqk", [C, 40], bf16, kind="ExternalInput")
    bqk_d = nc.dram_tensor("bqk", [1, 40], bf16, kind="ExternalInput")
    wv_d = nc.dram_tensor("wv", [C, C], bf16, kind="ExternalInput")
    bv_d = nc.dram_tensor("bv", [1, C], bf16, kind="ExternalInput")
    gam_d = nc.dram_tensor("gamma", [1, 1], f32, kind="ExternalInput")
    amat_d = nc.dram_tensor("amat", [128, 8 * 24], bf16, kind="ExternalInput")
    eye_d = nc.dram_tensor("eye", [128, 128], bf16, kind="ExternalInput")
    bmat_d = nc.dram_tensor("bmat", [128, W], bf16, kind="ExternalInput")
    y_d = nc.dram_tensor("y", [C, H * W], bf16, kind="ExternalOutput")

    with tile.TileContext(nc) as tc:
        with (
            tc.tile_pool(name="xbig", bufs=1) as xbig,
            tc.tile_pool(name="persist", bufs=1) as persist,
        ):
            x0 = xbig.tile([128, H * W], bf16)
            x1 = xbig.tile([128, H * W], bf16)
            xt = [x0, x1]

            # persistent tensors
            et_sb = persist.tile([128, 8, N], bf16)      # Et[n-chunk][n_l, m]
            vt_sb = persist.tile([128, 8, C + 1], bf16)  # Vt[n-chunk][n_l, c|1]
            o_sb = persist.tile([128, 8, C], bf16)        # O[m-chunk][m_l, c]
            rec_sb = persist.tile([128, 8], f32)
            recg_sb = persist.tile([128, 8], f32)
            a_sb = persist.tile([128, 8 * 24], bf16)
            b_sb = persist.tile([128, W], bf16)
            gam_sb = persist.tile([128, 1], f32)
            eye_sb = persist.tile([128, 128], bf16)
            wqk_sb = persist.tile([128, 2, 40], bf16)
            bqk_sb = persist.tile([1, 40], bf16)
            wv_sb = persist.tile([128, 2, C], bf16)
            bv_sb = persist.tile([1, C], bf16)
            q_sb = persist.tile([CQ, N], bf16)
            k_sb = persist.tile([CQ, N], bf16)


            arena_pool_cm = tc.tile_pool(name="arena", bufs=8)
            arenas = arena_pool_cm.__enter__()
            arena = {}

            def gen_arena(kc, eng, split=1):
                r0 = max(0, 16 * kc - 4)
                r1 = min(128, 16 * kc + 20)
                cnt = r1 - r0
                t_ = arenas.tile([128, 24, W], bf16, tag="arena")
                a32 = arenas.tile([128, 24, 32], bf16, tag="a32")
                aa = a_sb[:, kc * 24:kc * 24 + cnt]
                a_bc = bass.AP(tensor=aa.tensor, offset=aa.offset,
                               ap=[aa.ap[0], aa.ap[1], [0, 32]])
                nc.scalar.copy(a32[:, 0:cnt, :], a_bc)
                # all operands keep a packed [1, 32] last dim -> DVE 2x mode
                step = (cnt + split - 1) // split
                for j0 in range(0, cnt, step):
                    j1 = min(cnt, j0 + step)
                    t4 = t_[:, j0:j1, :].rearrange(
                        "p r (w4 w32) -> p r w4 w32", w32=32)
                    a4 = a32[:, j0:j1, :]
                    a_bc4 = bass.AP(tensor=a4.tensor, offset=a4.offset,
                                    ap=[a4.ap[0], a4.ap[1], [0, 4], a4.ap[2]])
                    bb = b_sb[:]
                    b_bc4 = bass.AP(tensor=bb.tensor, offset=bb.offset,
                                    ap=[bb.ap[0], [0, j1 - j0], [32, 4],
                                        [1, 32]])
                    eng.tensor_mul(t4, b_bc4, a_bc4)
                arena[kc] = (t_, r0)

            def slice_chunks(sl_i):
                hbs = {hb for hb in (sl_i - 1, sl_i, sl_i + 1) if 0 <= hb < 32}
                return sorted({hb // 4 for hb in hbs})

            with (
                tc.tile_pool(name="phase1", bufs=1) as ph1,
                tc.tile_pool(name="ptmp", bufs=1) as ptmp,
                tc.tile_pool(name="ps_qk", bufs=1, space="PSUM") as ps_qk,
                tc.tile_pool(name="ps_lt", bufs=1, space="PSUM") as ps_lt,
                tc.tile_pool(name="ps_vt", bufs=1, space="PSUM") as ps_vt,
                tc.tile_pool(name="ps_o", bufs=2, space="PSUM") as ps_o,
                tc.tile_pool(name="ps_y", bufs=3, space="PSUM") as ps_y,
            ):
                xf_sb = ph1.tile([128, 2, N], bf16)
                ones_sb = ph1.tile([1, N], bf16)
                nc.gpsimd.memset(ones_sb[:], 1.0)
                nc.gpsimd.memset(vt_sb[:, :, C:C + 1], 1.0)

                def do_qk(st):
                    sl = bass.ds(st * 128, 128)
                    qk_ps = ps_qk.tile([40, 128], f32, tag="qk")
                    nc.tensor.matmul(qk_ps[:], wqk_sb[:, 0, :],
                                     xf_sb[:, 0, sl], start=True, stop=False)
                    nc.tensor.matmul(qk_ps[:], wqk_sb[:, 1, :],
                                     xf_sb[:, 1, sl], start=False, stop=False)
                    nc.tensor.matmul(qk_ps[:], bqk_sb[:],
                                     ones_sb[:, sl], start=False, stop=True)
                    if st == 7:
                        nc.scalar.copy(k_sb[:, sl], qk_ps[32:40, :])
                        nc.vector.tensor_copy(q_sb[:, sl], qk_ps[0:CQ, :])
                    else:
                        nc.scalar.copy(q_sb[:, sl], qk_ps[0:CQ, :])
                        nc.scalar.copy(k_sb[:, sl], qk_ps[32:40, :])

                def do_vt(nk):
                    nsl = bass.ds(nk * 128, 128)
                    vt_ps = ps_vt.tile([128, C], f32, tag="vt")
                    nc.tensor.matmul(vt_ps[:], xf_sb[:, 0, nsl],
                                     wv_sb[:, 0, :], start=True, stop=False)
                    nc.tensor.matmul(vt_ps[:], xf_sb[:, 1, nsl],
                                     wv_sb[:, 1, :], start=False, stop=False)
                    nc.tensor.matmul(vt_ps[:], ones_sb[:, nsl],
                                     bv_sb[:], start=False, stop=True)
                    if nk == 7:
                        # keep ACT free for the critical exp chain
                        nc.vector.tensor_copy(vt_sb[:, nk, 0:C], vt_ps[:])
                    else:
                        nc.scalar.copy(vt_sb[:, nk, 0:C], vt_ps[:])

                def do_logits(nk, ms):
                    nsl = bass.ds(nk * 128, 128)
                    sl = bass.ds(ms * 512, 512)
                    lt_ps = ps_lt.tile([128, 512], f32, tag="lt")
                    nc.tensor.matmul(lt_ps[:], k_sb[:, nsl], q_sb[:, sl],
                                     start=True, stop=True)
                    nc.scalar.activation(et_sb[:, nk, sl], lt_ps[:], func=AF.Exp)

                o_ps_map = {}

                def attnv_mm(mk, nks, start):
                    if mk not in o_ps_map:
                        o_ps_map[mk] = ps_o.tile([128, C + 1], f32, tag="o",
                                                name=f"o_ps_{mk}")
                    o_ps = o_ps_map[mk]
                    msl = bass.ds(mk * 128, 128)
                    for i, nk in enumerate(nks):
                        nc.tensor.matmul(o_ps[:], et_sb[:, nk, msl],
                                         vt_sb[:, nk, :],
                                         start=(start and i == 0),
                                         stop=(nk == 7))

                def do_attnv_fin(mk):
                    o_ps = o_ps_map.pop(mk)
                    nc.vector.reciprocal(rec_sb[:, mk:mk + 1], o_ps[:, C:C + 1])
                    if mk == 0:
                        nc.vector.tensor_scalar(
                            out=o_sb[:, mk, :], in0=o_ps[:, 0:C],
                            scalar1=rec_sb[:, mk:mk + 1],
                            scalar2=gam_sb[:, 0:1],
                            op0=AL.mult, op1=AL.mult)
                    else:
                        nc.vector.tensor_scalar_mul(recg_sb[:, mk:mk + 1],
                                                    rec_sb[:, mk:mk + 1],
                                                    gam_sb[:, 0:1])
                        nc.scalar.mul(o_sb[:, mk, :], o_ps[:, 0:C],
                                      mul=recg_sb[:, mk:mk + 1])

                def do_attnv(mk, nks=range(8), start=True):
                    attnv_mm(mk, nks, start)
                    do_attnv_fin(mk)

                gstate = {"gi": 0}

                def do_groups(mk):
                    while gstate["gi"] < 32 and \
                            min(31, gstate["gi"] + 1) // 4 <= mk:
                        s_i = gstate["gi"]
                        ks = slice_chunks(s_i)
                        for kc in ks:
                            if kc not in arena:
                                gen_arena(kc, nc.vector)
                        for ch in range(2):
                            lane = (s_i * 2 + ch) % 8
                            on_act = lane % 2 == 0
                            osl = bass.ds(s_i * 512, 512)
                            y_ps = ps_y.tile([128, 512], f32, tag="y")
                            # bilinear rows: r0,r1 draw from hb s_i-1,s_i;
                            # r2,r3 from hb s_i,s_i+1 -> a neighbouring kc
                            # chunk only feeds half the tile's columns.
                            if len(ks) == 1:
                                parts = [(ks[0], 0, 4)]
                            elif s_i % 4 == 0:
                                parts = [(ks[1], 0, 4), (ks[0], 0, 2)]
                            else:
                                parts = [(ks[0], 0, 4), (ks[1], 2, 4)]
                            for i, (kc, ra, rb) in enumerate(parts):
                                t_, r0 = arena[kc]
                                nc.tensor.matmul(
                                    y_ps[:, ra * 128:rb * 128],
                                    o_sb[:, kc, ch * 128:(ch + 1) * 128],
                                    t_[:, 4 * s_i - r0 + ra:4 * s_i - r0 + rb,
                                       :],
                                    start=(i == 0),
                                    stop=(not on_act and
                                          i == len(parts) - 1))
                            if on_act:
                                nc.tensor.matmul(
                                    y_ps[:], eye_sb[:], xt[ch][:, osl],
                                    start=False, stop=True)
                                nc.scalar.copy(xt[ch][:, osl], y_ps[:])
                            else:
                                nc.vector.tensor_add(xt[ch][:, osl], y_ps[:],
                                                     xt[ch][:, osl])
                            if s_i < 4:
                                nc.sync.dma_start(
                                    out=y_d[ch * 128:(ch + 1) * 128, osl],
                                    in_=xt[ch][:, osl])
                            elif s_i < 8:
                                if s_i % 2 == 1:
                                    dsl = bass.ds((s_i - 1) * 512, 1024)
                                    nc.sync.dma_start(
                                        out=y_d[ch * 128:(ch + 1) * 128, dsl],
                                        in_=xt[ch][:, dsl])
                            elif s_i < 16:
                                if s_i % 2 == 1:
                                    dsl = bass.ds((s_i - 1) * 512, 1024)
                                    nc.sync.dma_start(
                                        out=y_d[ch * 128:(ch + 1) * 128, dsl],
                                        in_=xt[ch][:, dsl])
                            elif s_i % 4 == 3:
                                dsl = bass.ds((s_i - 3) * 512, 2048)
                                nc.sync.dma_start(
                                    out=y_d[ch * 128:(ch + 1) * 128, dsl],
                                    in_=xt[ch][:, dsl])
                        gstate["gi"] += 1

                # ---- streamed input phase ----
                for st in range(8):
                    if st == 7:
                        for hf in range(2):
                            for t in range(2):
                                sl = bass.ds(st * 2048 + hf * 1024, 1024)
                                nc.sync.dma_start(
                                    out=xt[t][:, sl],
                                    in_=x_d[t * 128:(t + 1) * 128, sl])
                    else:
                        for t in range(2):
                            sl = bass.ds(st * 2048, 2048)
                            nc.sync.dma_start(out=xt[t][:, sl],
                                              in_=x_d[t * 128:(t + 1) * 128, sl])
                    def pool_strip(t, off, ln, wres, s2eng=None):
                        nh = ln // 512
                        strip = xt[t][:, bass.ds(st * 2048 + off, ln)]
                        v1 = strip.rearrange("p (h two w) -> p h two w",
                                             two=2, w=128)
                        t1 = ptmp.tile([128, nh * 2, 128], bf16,
                                       tag=f"t1_{t}_{off}_{ln}",
                                       name=f"t1_{t}_{off}_{ln}")
                        nc.vector.tensor_add(t1[:], v1[:, :, 0, :],
                                             v1[:, :, 1, :])
                        v2 = t1[:].rearrange("p (h two) w -> p h two w",
                                             two=2)
                        t2 = ptmp.tile([128, nh, 128], bf16,
                                       tag=f"t2_{t}_{off}_{ln}",
                                       name=f"t2_{t}_{off}_{ln}")
                        (s2eng or nc.vector).tensor_add(t2[:], v2[:, :, 0, :],
                                                        v2[:, :, 1, :])
                        v3 = t2[:].rearrange("p hb (wp two) -> p hb wp two",
                                             two=2)
                        t3 = ptmp.tile([128, nh, 64], bf16,
                                       tag=f"t3_{t}_{off}_{ln}",
                                       name=f"t3_{t}_{off}_{ln}")
                        wres.tensor_add(t3[:], v3[:, :, :, 0],
                                        v3[:, :, :, 1])
                        v4 = t3[:].rearrange("p hb (wb two) -> p hb wb two",
                                             two=2)
                        xfs = xf_sb[:, t,
                                    bass.ds(st * 128 + off // 16, ln // 16)
                                    ].rearrange("p (hb wb) -> p hb wb",
                                                hb=nh)
                        wres.tensor_add(xfs, v4[:, :, :, 0],
                                        v4[:, :, :, 1])

                    if st == 0:
                        nc.sync.dma_start(
                            out=wv_sb[:],
                            in_=wv_d[:].rearrange("(t p) o -> p t o", p=128))
                        nc.sync.dma_start(out=bv_sb[:], in_=bv_d[:])
                        nc.sync.dma_start(
                            out=wqk_sb[:],
                            in_=wqk_d[:].rearrange("(t p) o -> p t o", p=128))
                        nc.sync.dma_start(out=bqk_sb[:], in_=bqk_d[:])
                    if st == 1:
                        nc.sync.dma_start(out=eye_sb[:], in_=eye_d[:])
                        nc.sync.dma_start(out=a_sb[:], in_=amat_d[:])
                        nc.sync.dma_start(out=b_sb[:], in_=bmat_d[:])
                        nc.sync.dma_start(out=gam_sb[:],
                                          in_=gam_d[:].to_broadcast((128, 1)))
                    if st == 2:
                        gen_arena(0, nc.gpsimd)
                        gen_arena(1, nc.vector)
                    if st == 4:
                        gen_arena(2, nc.vector)
                    if st != 7:
                        pool_strip(0, 0, 2048, nc.gpsimd)
                        pool_strip(1, 0, 2048, nc.gpsimd)
                        do_vt(st)
                        do_qk(st)
                    else:
                        # tail: column-split logits so attnV mk0 only waits
                        # on the first 128 m-columns. All writes keep full
                        # partition dim (partition-sliced writes race with
                        # cross-engine readers under the tile scheduler).
                        pool_strip(0, 0, 1024, nc.gpsimd)
                        pool_strip(1, 0, 1024, nc.gpsimd)
                        pool_strip(0, 1024, 1024, nc.gpsimd)
                        pool_strip(1, 1024, 1024, nc.vector)
                        do_vt(7)
                        do_qk(7)
                        nsl = bass.ds(7 * 128, 128)
                        lt7 = ps_lt.tile([128, 128], f32, tag="lt")
                        nc.tensor.matmul(lt7[:], k_sb[:, nsl], q_sb[:, 0:128],
                                         start=True, stop=True)
                        nc.scalar.activation(et_sb[:, 7, 0:128], lt7[:],
                                             func=AF.Exp)
                        lt_b = ps_lt.tile([128, 384], f32, tag="lt")
                        nc.tensor.matmul(lt_b[:], k_sb[:, nsl],
                                         q_sb[:, 128:512],
                                         start=True, stop=True)
                        nc.scalar.activation(et_sb[:, 7, 128:512], lt_b[:],
                                             func=AF.Exp)

                    if st == 3:
                        for nk in range(4):
                            do_logits(nk, 0)
                    if 4 <= st <= 6:
                        do_logits(st, 0)
                    if st == 5:
                        attnv_mm(0, [0, 1, 2, 3], True)
                        attnv_mm(1, [0, 1, 2, 3], True)
                    if st == 6:
                        attnv_mm(0, [4, 5, 6], False)
                        attnv_mm(1, [4, 5, 6], False)

                # ---- attention output / upsample, m-half 0 first ----
                do_attnv(0, [7], start=False)
                do_attnv(1, [7], start=False)
                do_groups(0)
                do_attnv(2)
                do_groups(1)
                do_logits(0, 1)
                do_logits(1, 1)
                do_logits(2, 1)
                do_logits(3, 1)
                do_attnv(3)
                do_groups(2)
                gen_arena(3, nc.vector, split=2)
                gen_arena(4, nc.vector, split=2)
                do_logits(4, 1)
                do_logits(5, 1)
                do_logits(6, 1)
                do_logits(7, 1)
                do_groups(3)
                gen_arena(5, nc.vector, split=2)
                gen_arena(6, nc.vector, split=2)
                do_attnv(4)
                do_attnv(5)
                do_groups(4)
                gen_arena(7, nc.vector, split=2)
                do_attnv(6)
                do_groups(5)
                do_attnv(7)
                do_groups(6)
                do_groups(7)
            arena_pool_cm.__exit__(None, None, None)
    nc.compile()
    return nc


def _get_nc():
    if "nc" not in _CACHE:
        _CACHE["nc"] = _build_bass()
    return _CACHE["nc"]


def kernel(x, Wq, bq, Wk, bk, Wv, bv, gamma):
    from concourse.bass_utils import run_bass_kernel_spmd

    x = np.ascontiguousarray(np.asarray(x, dtype=np.float32))
    U = _resize_matrix(H, HD)  # [128, 32]

    import ml_dtypes as _mld
    p = np.arange(128)
    amat = np.zeros((128, 8 * 24), dtype=_mld.bfloat16)
    for kc in range(8):
        r0 = max(0, 16 * kc - 4)
        r1 = min(128, 16 * kc + 20)
        full = U[:, 4 * kc + p // 32].T.astype(_mld.bfloat16)  # [128p, 128H]
        amat[:, kc * 24:kc * 24 + (r1 - r0)] = full[:, r0:r1]
    bmat = np.ascontiguousarray(U[:, p % 32].T.astype(_mld.bfloat16))
    eye = np.eye(128, dtype=_mld.bfloat16)

    import ml_dtypes
    bfd = ml_dtypes.bfloat16
    wqk = np.zeros((C, 40), dtype=bfd)
    wqk[:, 0:8] = (np.asarray(Wq).T / 16.0).astype(bfd)
    wqk[:, 32:40] = (np.asarray(Wk).T / 16.0).astype(bfd)
    bqk = np.zeros((1, 40), dtype=bfd)
    bqk[0, 0:8] = np.asarray(bq).astype(bfd)
    bqk[0, 32:40] = np.asarray(bk).astype(bfd)
    wv = np.ascontiguousarray(np.asarray(Wv).T / 16.0).astype(bfd)
    bvr = np.asarray(bv)[None, :].astype(bfd)
    gam = np.asarray(gamma).reshape(1, 1).astype(np.float32)

    nc = _get_nc()
    in_maps = []
    for i in range(NCORES):
        in_maps.append({
            "x": np.ascontiguousarray(x[i].reshape(C, H * W)).astype(bfd),
            "wqk": wqk.copy(), "bqk": bqk.copy(), "wv": wv.copy(),
            "bv": bvr.copy(), "gamma": gam.copy(), "amat": amat.copy(),
            "bmat": bmat.copy(), "eye": eye.copy(),
        })
    res = run_bass_kernel_spmd(nc, in_maps, core_ids=list(range(NCORES)))
    y = np.stack([np.asarray(r["y"], dtype=np.float32).reshape(C, H, W)
                  for r in res.results])
    return y


if __name__ == "__main__":
    rng = np.random.default_rng(0)
    inputs = {
        "x": rng.standard_normal((B, C, H, W), dtype=np.float32),
        "Wq": rng.standard_normal((CQ, C), dtype=np.float32) * 0.05,
        "bq": rng.standard_normal((CQ,), dtype=np.float32) * 0.05,
        "Wk": rng.standard_normal((CQ, C), dtype=np.float32) * 0.05,
        "bk": rng.standard_normal((CQ,), dtype=np.float32) * 0.05,
        "Wv": rng.standard_normal((C, C), dtype=np.float32) * 0.05,
        "bv": rng.standard_normal((C,), dtype=np.float32) * 0.05,
        "gamma": np.zeros((1,), dtype=np.float32),
    }
    y = kernel(**inputs)
    print("out", y.shape, y.dtype, float(np.abs(y - inputs["x"]).max()))



# revision 5
# speedup vs baseline: 1.0088x; 1.0088x over previous
"""Trainium2 Bass kernel for the AttentionBlock problem (v3).

Math (per batch b, one NeuronCore each):
  x_down = avgpool4x4(x)            # [C, 32, 32] -> xf [C, N], N=1024
  q,k = Wq/Wk @ xf + b              # [8, N]
  v = Wv @ xf + bv                  # [C, N]
  attn = softmax_n(q^T k)           # [N, N]
  out[c,m] = sum_n v[c,n] attn[m,n]
  y = gamma * upsample_bilinear(out) + x

v3 design (cost-model driven):
  - DMA is the serial floor: 8.4MB in + 8.4MB out bf16 ~= 46.6us. Everything
    else is hidden under it: pooling + projections + m-half-0 logits during
    the input stream, a ~3us bridge, then a DMA-paced output stream.
  - All attention-branch matmuls run in fp8e4m3 with DoubleRow perf mode
    (0.5 cyc/col, 256-deep contraction): v/q/k projections contract the two
    c-halves at once; attn@V contracts n-chunk pairs; the fused
    upsample+gamma matmul contracts two m-chunks at once. The residual path
    stays bf16, so with gamma==0 the output is exactly the bf16 roundtrip
    of x.
  - Bilinear row weights repeat every 4 slices, so 6 static "universal
    slabs" [128m, 2kt, 4r x 128w] (4 interior phases + 2 edges) replace
    per-slice weight generation. They are built on-device from a tiny
    host table A (x gamma) and the W-resize matrix B: slab = A*B, fp8 out.
    A dummy zero chunk on each end of o_sb keeps every slice a contiguous
    chunk pair.
  - Output phase per si-pair (1024 cols, 2 c-halves): 2 fp8 up-matmuls per
    half; ch0 drains on DVE (scalar_tensor_tensor: psum + gamma*bv + x),
    ch1 folds x into PSUM via an identity matmul and drains on ACT
    (Identity activation with per-partition gamma*bv bias). Softmax
    denominators ride as a ones-column in Vt; reciprocals are batched in
    mk pairs; rec-scale happens in the PSUM->SBUF fp8 copy of O.
  - q/k biases are applied by the ACT copy (per-partition bias + 1/8 scale
    undoing the fp8 weight scaling); bv is applied as the per-partition
    drain bias (bilinear rows sum to 1); 1/16 pooling mean lives in the
    xf->fp8 convert; Wv's 4x fp8 scaling is undone inside the slab table.
"""

import numpy as np

B, C, H, W = 8, 256, 128, 128
HD, WD = 32, 32
N = HD * WD  # 1024
CQ = 8
NCORES = 8

_CACHE = {}


def _resize_matrix(dst: int, src: int) -> np.ndarray:
    """Bilinear (half-pixel, edge-renormalized) resize matrix, matches
    jax.image.resize(method='linear') for upsampling."""
    scale = dst / src
    pos = (np.arange(dst, dtype=np.float64) + 0.5) / scale - 0.5
    j = np.arange(src, dtype=np.float64)
    w = np.maximum(0.0, 1.0 - np.abs(pos[:, None] - j[None, :]))
    w = w / w.sum(axis=1, keepdims=True)
    return w.astype(np.float32)  # [dst, src]


def _slab_id(si):
    if si == 0:
        return 4
    if si == 31:
        return 5
    return si % 4


def _pair_lo_mk(si):
    a = si // 4
    if si == 0:
        return -1  # (dummy, mk0)
    if si % 4 == 0:
        return a - 1
    if si >= 29:
        return 7
    return a


def _slab_tables():
    """A[p, s, kt*4+r] H-weight table for the 6 universal slabs and the
    per-si (slab id, o_sb dev-chunk lo) map. dev chunk = mk + 1 with dummy
    zero chunks at 0 and 9."""
    UH = _resize_matrix(H, HD)
    reps = {}
    for si in range(32):
        reps.setdefault(_slab_id(si), si)
    A = np.zeros((128, 6, 8), np.float32)
    for s, si in reps.items():
        lo = _pair_lo_mk(si)
        for kt in range(2):
            mk = lo + kt
            if not (0 <= mk <= 7):
                continue
            for sub in range(4):
                hb = mk * 4 + sub
                if abs(hb - si) <= 1 and 0 <= hb < 32:
                    for r in range(4):
                        A[sub * 32:(sub + 1) * 32, s, kt * 4 + r] = \
                            UH[4 * si + r, hb]
    # universality check: the rep-si table must reproduce every si exactly
    for si in range(32):
        s, lo = _slab_id(si), _pair_lo_mk(si)
        for kt in range(2):
            mk = lo + kt
            for sub in range(4):
                hb = mk * 4 + sub
                want = (UH[4 * si:4 * si + 4, hb] if 0 <= mk <= 7 and hb < 32
                        else np.zeros(4, np.float32))
                got = A[sub * 32, s, kt * 4:kt * 4 + 4]
                assert np.allclose(got, want), (si, kt, sub, got, want)
    simap = [(_slab_id(si), _pair_lo_mk(si) + 1) for si in range(32)]
    return A, simap


_A_TABLE, _SIMAP = _slab_tables()


def _build_bass():
    import concourse.bass as bass
    import concourse.tile as tile
    from concourse import bacc, mybir

    f32 = mybir.dt.float32
    bf16 = mybir.dt.bfloat16
    fp8 = mybir.dt.float8e4
    AF = mybir.ActivationFunctionType
    AL = mybir.AluOpType
    DR = mybir.MatmulPerfMode.DoubleRow

    nc = bacc.Bacc("TRN2", target_bir_lowering=False, debug=False)

    x_d = nc.dram_tensor("x", [C, H * W], bf16, kind="ExternalInput")
    wqk_d = nc.dram_tensor("wqk8", [128, 2 * 40], fp8, kind="ExternalInput")
    bqk_d = nc.dram_tensor("bqk", [40, 1], f32, kind="ExternalInput")
    wv_d = nc.dram_tensor("wv8", [128, 2 * C], fp8, kind="ExternalInput")
    bvt_d = nc.dram_tensor("bvt", [128, 2], f32, kind="ExternalInput")
    gam_d = nc.dram_tensor("gamma", [1, 1], f32, kind="ExternalInput")
    amat_d = nc.dram_tensor("amat", [128, 6 * 8], bf16, kind="ExternalInput")
    bmat_d = nc.dram_tensor("bmat", [128, W], bf16, kind="ExternalInput")
    eye_d = nc.dram_tensor("eye", [128, 128], bf16, kind="ExternalInput")
    y_d = nc.dram_tensor("y", [C, H * W], bf16, kind="ExternalOutput")

    with tile.TileContext(nc) as tc:
        with (
            tc.tile_pool(name="xbig", bufs=1) as xbig,
            tc.tile_pool(name="persist", bufs=1) as persist,
            tc.tile_pool(name="ptmp", bufs=1) as ptmp,
            tc.tile_pool(name="ps_lt", bufs=1, space="PSUM") as ps_lt,
            tc.tile_pool(name="ps_o", bufs=1, space="PSUM") as ps_o,
        ):
            x0 = xbig.tile([128, H * W], bf16)
            x1 = xbig.tile([128, H * W], bf16)
            xt = [x0, x1]

            et_sb = persist.tile([128, 8, N], fp8)       # Et[nk][n_l, m]
            vt_sb = persist.tile([128, 8, C + 1], fp8)   # Vt[nk][n_l, c|1]
            o_sb = persist.tile([128, 10, C], fp8)       # O[dev mk][m_l, c]
            slab_sb = persist.tile([128, 6, 2, 4, W], fp8)
            xf_sb = persist.tile([128, 2, N], bf16)
            xf8_sb = persist.tile([128, 2, N], fp8)
            q_sb = persist.tile([CQ, N], bf16)
            k_sb = persist.tile([CQ, N], bf16)
            rec_sb = persist.tile([128, 8], f32)
            wqk_sb = persist.tile([128, 2, 40], fp8)
            bqk_sb = persist.tile([40, 1], f32)
            wv_sb = persist.tile([128, 2, C], fp8)
            bvt_sb = persist.tile([128, 2], f32)
            gbv_sb = persist.tile([128, 2], f32)
            gam_sb = persist.tile([128, 1], f32)
            am_sb = persist.tile([128, 6, 8], bf16)
            ag_sb = persist.tile([128, 6, 8], bf16)
            a32_sb = persist.tile([128, 6, 8, 32], bf16)
            b_sb = persist.tile([128, W], bf16)
            eye_sb = persist.tile([128, 128], bf16)

            ps_qkvt_cm = tc.tile_pool(name="ps_qkvt", bufs=1, space="PSUM")
            ps_qkvt = ps_qkvt_cm.__enter__()

            # ---------- helpers ----------
            def pool_strip(t, st, off, ln):
                nh = ln // 512
                strip = xt[t][:, bass.ds(st * 2048 + off, ln)]
                v1 = strip.rearrange("p (h two w) -> p h two w", two=2, w=128)
                t1 = ptmp.tile([128, nh * 2, 128], bf16,
                               tag=f"t1_{t}_{off}_{ln}",
                               name=f"t1_{t}_{off}_{ln}")
                nc.vector.tensor_add(t1[:], v1[:, :, 0, :], v1[:, :, 1, :])
                v2 = t1[:].rearrange("p (h two) w -> p h two w", two=2)
                t2 = ptmp.tile([128, nh, 128], bf16,
                               tag=f"t2_{t}_{off}_{ln}",
                               name=f"t2_{t}_{off}_{ln}")
                nc.vector.tensor_add(t2[:], v2[:, :, 0, :], v2[:, :, 1, :])
                v3 = t2[:].rearrange("p hb (wp two) -> p hb wp two", two=2)
                t3 = ptmp.tile([128, nh, 64], bf16,
                               tag=f"t3_{t}_{off}_{ln}",
                               name=f"t3_{t}_{off}_{ln}")
                nc.gpsimd.tensor_add(t3[:], v3[:, :, :, 0], v3[:, :, :, 1])
                v4 = t3[:].rearrange("p hb (wb two) -> p hb wb two", two=2)
                xfs = xf_sb[:, t, bass.ds(st * 128 + off // 16, ln // 16)
                            ].rearrange("p (hb wb) -> p hb wb", hb=nh)
                nc.gpsimd.tensor_add(xfs, v4[:, :, :, 0], v4[:, :, :, 1])

            def xf8_conv(c0, ln):
                sl = bass.ds(c0, ln)
                nc.gpsimd.tensor_scalar(out=xf8_sb[:, :, sl],
                                        in0=xf_sb[:, :, sl],
                                        scalar1=1.0 / 16.0, scalar2=None,
                                        op0=AL.mult)

            def do_vt(nk):
                nsl = bass.ds(nk * 128, 128)
                vt_ps = ps_qkvt.tile([128, C], f32, tag="vt", name="vt_ps")
                nc.tensor.matmul(vt_ps[:], xf8_sb[:, :, nsl], wv_sb[:],
                                 start=True, stop=True, perf_mode=DR)
                nc.scalar.copy(vt_sb[:, nk, 0:C], vt_ps[:])

            def do_qk(st):
                nsl = bass.ds(st * 128, 128)
                qk_ps = ps_qkvt.tile([40, 128], f32, tag="qk", name="qk_ps")
                nc.tensor.matmul(qk_ps[:], wqk_sb[:], xf8_sb[:, :, nsl],
                                 start=True, stop=True, perf_mode=DR)
                nc.scalar.activation(q_sb[:, nsl], qk_ps[0:CQ, :],
                                     func=AF.Identity, bias=bqk_sb[0:CQ, :],
                                     scale=0.125)
                nc.scalar.activation(k_sb[:, nsl], qk_ps[32:40, :],
                                     func=AF.Identity, bias=bqk_sb[32:40, :],
                                     scale=0.125)

            def do_logits(nk, m0, mlen, lt=None):
                nsl = bass.ds(nk * 128, 128)
                if lt is None:
                    lt = ps_lt.tile([128, 512], f32, tag="lt", name="lt_ps")
                nc.tensor.matmul(lt[:, 0:mlen], k_sb[:, nsl],
                                 q_sb[:, bass.ds(m0, mlen)],
                                 start=True, stop=True)
                nc.scalar.activation(et_sb[:, nk, bass.ds(m0, mlen)],
                                     lt[:, 0:mlen], func=AF.Exp)
                return lt

            o_ps_holder = {}

            def attnv_mm(oj, mk, pks, start):
                o_ps = o_ps_holder["t"]
                msl = bass.ds(mk * 128, 128)
                for i, pk in enumerate(pks):
                    nc.tensor.matmul(o_ps[:, oj, 0:C + 1],
                                     et_sb[:, pk:pk + 2, msl],
                                     vt_sb[:, pk:pk + 2, :],
                                     start=(start and i == 0),
                                     stop=(pk == 6), perf_mode=DR)

            def attnv_fin(mk0):
                # mk0, mk0+1 live in o_ps halves 0/1; batch the recip.
                o_ps = o_ps_holder["t"]
                nc.vector.reciprocal(rec_sb[:, mk0:mk0 + 2], o_ps[:, :, C])
                nc.vector.tensor_scalar(out=o_sb[:, mk0 + 1, :],
                                        in0=o_ps[:, 0, 0:C],
                                        scalar1=rec_sb[:, mk0:mk0 + 1],
                                        scalar2=None, op0=AL.mult)
                nc.scalar.mul(o_sb[:, mk0 + 2, :], o_ps[:, 1, 0:C],
                              mul=rec_sb[:, mk0 + 1:mk0 + 2])

            def gen_slab(s):
                aa = ag_sb[:, s, :]
                a_bc = bass.AP(tensor=aa.tensor, offset=aa.offset,
                               ap=[aa.ap[0], aa.ap[1], [0, 32]])
                nc.scalar.copy(a32_sb[:, s], a_bc)
                out4 = slab_sb[:, s].rearrange("p kt r (wq w) -> p kt r wq w",
                                               w=32)
                bb = b_sb[:]
                b_bc = bass.AP(tensor=bb.tensor, offset=bb.offset,
                               ap=[bb.ap[0], [0, 2], [0, 4], [32, 4], [1, 32]])
                a4 = a32_sb[:, s]
                a_bc4 = bass.AP(tensor=a4.tensor, offset=a4.offset,
                                ap=[a4.ap[0], [128, 2], [32, 4], [0, 4],
                                    [1, 32]])
                nc.vector.tensor_mul(out4, b_bc, a_bc4)

            # ---------- input phase ----------
            for st in range(8):
                if st == 7:
                    for hf in range(2):
                        for t in range(2):
                            sl = bass.ds(st * 2048 + hf * 1024, 1024)
                            nc.sync.dma_start(
                                out=xt[t][:, sl],
                                in_=x_d[t * 128:(t + 1) * 128, sl])
                else:
                    for t in range(2):
                        sl = bass.ds(st * 2048, 2048)
                        nc.sync.dma_start(out=xt[t][:, sl],
                                          in_=x_d[t * 128:(t + 1) * 128, sl])
                if st == 0:
                    nc.sync.dma_start(
                        out=wqk_sb[:],
                        in_=wqk_d[:].rearrange("p (k m) -> p k m", k=2))
                    nc.sync.dma_start(out=bqk_sb[:], in_=bqk_d[:])
                    nc.sync.dma_start(
                        out=wv_sb[:],
                        in_=wv_d[:].rearrange("p (k m) -> p k m", k=2))
                    nc.sync.dma_start(out=bvt_sb[:], in_=bvt_d[:])
                    nc.sync.dma_start(out=gam_sb[:],
                                      in_=gam_d[:].to_broadcast((128, 1)))
                    nc.sync.dma_start(
                        out=am_sb[:],
                        in_=amat_d[:].rearrange("p (s r) -> p s r", s=6))
                    nc.sync.dma_start(out=b_sb[:], in_=bmat_d[:])
                    nc.sync.dma_start(out=eye_sb[:], in_=eye_d[:])
                    nc.gpsimd.memset(vt_sb[:, :, C:C + 1], 1.0)
                    nc.gpsimd.memset(o_sb[:, 0, :], 0.0)
                    nc.gpsimd.memset(o_sb[:, 9, :], 0.0)
                if st == 1:
                    nc.vector.tensor_scalar(out=ag_sb[:], in0=am_sb[:],
                                            scalar1=gam_sb[:, 0:1],
                                            scalar2=None, op0=AL.mult)
                    nc.vector.tensor_scalar(out=gbv_sb[:], in0=bvt_sb[:],
                                            scalar1=gam_sb[:, 0:1],
                                            scalar2=None, op0=AL.mult)

                if st != 7:
                    pool_strip(0, st, 0, 2048)
                    pool_strip(1, st, 0, 2048)
                    xf8_conv(st * 128, 128)
                    do_vt(st)
                    do_qk(st)
                else:
                    pool_strip(0, st, 0, 1024)
                    pool_strip(1, st, 0, 1024)
                    xf8_conv(st * 128, 64)
                    pool_strip(0, st, 1024, 1024)
                    pool_strip(1, st, 1024, 1024)
                    xf8_conv(st * 128 + 64, 64)
                    do_vt(7)
                    do_qk(7)

                if 1 <= st <= 6:
                    gen_slab(st - 1)
                if st == 3:
                    for nk in range(4):
                        do_logits(nk, 0, 512)
                if 4 <= st <= 6:
                    do_logits(st, 0, 512)
                if st == 4:
                    o_ps_holder["t"] = ps_o.tile([128, 2, 512], f32, tag="o",
                                                 name="o_ps")
                    attnv_mm(0, 0, [0, 2], True)
                    attnv_mm(1, 1, [0, 2], True)
                if st == 6:
                    attnv_mm(0, 0, [4], False)
                    attnv_mm(1, 1, [4], False)

            # ---------- bridge ----------
            nsl7 = bass.ds(7 * 128, 128)
            lt7 = ps_lt.tile([128, 512], f32, tag="lt", name="lt_ps")
            nc.tensor.matmul(lt7[:, 0:128], k_sb[:, nsl7],
                             q_sb[:, 0:128], start=True, stop=True)
            nc.scalar.activation(et_sb[:, 7, 0:128], lt7[:, 0:128],
                                 func=AF.Exp)
            nc.tensor.matmul(lt7[:, 128:512], k_sb[:, nsl7],
                             q_sb[:, 128:512], start=True, stop=True)
            nc.scalar.activation(et_sb[:, 7, 128:512], lt7[:, 128:512],
                                 func=AF.Exp)
            attnv_mm(0, 0, [6], False)
            attnv_mm(1, 1, [6], False)
            attnv_fin(0)

            ps_qkvt_cm.__exit__(None, None, None)
            ps_y_cm = tc.tile_pool(name="ps_y", bufs=2, space="PSUM")
            ps_y = ps_y_cm.__enter__()

            # ---------- output phase ----------
            def up_pair(p, ch):
                y_ps = ps_y.tile([128, 1024], f32, tag="y", name="y_ps")
                for j in (0, 1):
                    si = 2 * p + j
                    s, dlo = _SIMAP[si]
                    reg = y_ps[:, j * 512:(j + 1) * 512]
                    nc.tensor.matmul(
                        reg, o_sb[:, dlo:dlo + 2, ch * 128:(ch + 1) * 128],
                        slab_sb[:, s].rearrange("p kt r w -> p kt (r w)"),
                        start=True, stop=(ch == 0), perf_mode=DR)
                    if ch == 1:
                        nc.tensor.matmul(
                            reg, eye_sb[:],
                            xt[1][:, bass.ds(si * 512, 512)],
                            start=False, stop=True)
                psl = bass.ds(p * 1024, 1024)
                if ch == 0:
                    nc.vector.scalar_tensor_tensor(
                        out=xt[0][:, psl], in0=y_ps[:],
                        scalar=gbv_sb[:, 0:1], in1=xt[0][:, psl],
                        op0=AL.add, op1=AL.add)
                else:
                    nc.scalar.activation(xt[1][:, psl], y_ps[:],
                                         func=AF.Identity,
                                         bias=gbv_sb[:, 1:2], scale=1.0)
                nc.sync.dma_start(out=y_d[ch * 128:(ch + 1) * 128, psl],
                                  in_=xt[ch][:, psl])

            def side(p):
                if p == 1:
                    do_logits(0, 512, 512)
                    do_logits(1, 512, 512)
                elif p == 2:
                    attnv_mm(0, 2, [0, 2, 4, 6], True)
                    attnv_mm(1, 3, [0, 2, 4, 6], True)
                    attnv_fin(2)
                elif p == 3:
                    do_logits(2, 512, 512)
                    do_logits(3, 512, 512)
                elif p == 4:
                    do_logits(4, 512, 512)
                    do_logits(5, 512, 512)
                elif p == 5:
                    do_logits(6, 512, 512)
                    do_logits(7, 512, 512)
                elif p == 6:
                    attnv_mm(0, 4, [0, 2, 4, 6], True)
                    attnv_mm(1, 5, [0, 2, 4, 6], True)
                    attnv_fin(4)
                elif p == 10:
                    attnv_mm(0, 6, [0, 2, 4, 6], True)
                    attnv_mm(1, 7, [0, 2, 4, 6], True)
                    attnv_fin(6)

            for p in range(16):
                up_pair(p, 0)
                up_pair(p, 1)
                side(p)

            ps_y_cm.__exit__(None, None, None)
    nc.compile()
    return nc


def _get_nc():
    if "nc" not in _CACHE:
        _CACHE["nc"] = _build_bass()
    return _CACHE["nc"]


def kernel(x, Wq, bq, Wk, bk, Wv, bv, gamma):
    from concourse.bass_utils import run_bass_kernel_spmd
    import ml_dtypes

    bfd = ml_dtypes.bfloat16
    f8d = ml_dtypes.float8_e4m3

    x = np.ascontiguousarray(np.asarray(x, dtype=np.float32))
    UW = _resize_matrix(W, WD)  # [128, 32]

    p = np.arange(128)
    # B[p, w] = UW[w, p%32] * 0.25 (undoes the 4x fp8 scaling of Wv)
    bmat = np.ascontiguousarray(UW[:, p % 32].T * 0.25).astype(bfd)
    amat = np.ascontiguousarray(_A_TABLE.reshape(128, 48)).astype(bfd)
    eye = np.eye(128, dtype=bfd)

    wqk8 = np.zeros((128, 2, 40), dtype=f8d)
    Wqa = np.asarray(Wq, dtype=np.float32)
    Wka = np.asarray(Wk, dtype=np.float32)
    Wva = np.asarray(Wv, dtype=np.float32)
    for h in range(2):
        wqk8[:, h, 0:8] = (8.0 * Wqa[:, h * 128:(h + 1) * 128].T).astype(f8d)
        wqk8[:, h, 32:40] = (8.0 * Wka[:, h * 128:(h + 1) * 128].T
                             ).astype(f8d)
    bqk = np.zeros((40, 1), dtype=np.float32)
    bqk[0:8, 0] = np.asarray(bq, dtype=np.float32)
    bqk[32:40, 0] = np.asarray(bk, dtype=np.float32)
    wv8 = np.zeros((128, 2, C), dtype=f8d)
    for h in range(2):
        wv8[:, h, :] = (4.0 * Wva[:, h * 128:(h + 1) * 128].T).astype(f8d)
    bvt = np.ascontiguousarray(
        np.asarray(bv, dtype=np.float32).reshape(2, 128).T)
    gam = np.asarray(gamma).reshape(1, 1).astype(np.float32)

    nc = _get_nc()
    in_maps = []
    for i in range(NCORES):
        in_maps.append({
            "x": np.ascontiguousarray(x[i].reshape(C, H * W)).astype(bfd),
            "wqk8": np.ascontiguousarray(wqk8.reshape(128, 80)),
            "bqk": bqk.copy(),
            "wv8": np.ascontiguousarray(wv8.reshape(128, 2 * C)),
            "bvt": bvt.copy(),
            "gamma": gam.copy(),
            "amat": amat.copy(),
            "bmat": bmat.copy(),
            "eye": eye.copy(),
        })
    res = run_bass_kernel_spmd(nc, in_maps, core_ids=list(range(NCORES)))
    y = np.stack([np.asarray(r["y"], dtype=np.float32).reshape(C, H, W)
                  for r in res.results])
    return y


def _np_reference(x, Wq, bq, Wk, bk, Wv, bv, gamma):
    b, c, h, w = x.shape
    hd, wd = h // 4, w // 4
    xd = x.reshape(b, c, hd, 4, wd, 4).mean(axis=(3, 5))
    xf = xd.reshape(b, c, hd * wd)
    q = np.einsum('oc,bcn->bon', Wq, xf) + bq[None, :, None]
    k = np.einsum('oc,bcn->bon', Wk, xf) + bk[None, :, None]
    v = np.einsum('oc,bcn->bon', Wv, xf) + bv[None, :, None]
    lg = np.einsum('bcm,bcn->bmn', q, k)
    lg = np.exp(lg - lg.max(axis=-1, keepdims=True))
    attn = lg / lg.sum(axis=-1, keepdims=True)
    out = np.einsum('bcn,bmn->bcm', v, attn).reshape(b, c, hd, wd)
    UH = _resize_matrix(h, hd)
    UW = _resize_matrix(w, wd)
    up = np.einsum('hj,bcjk,wk->bchw', UH, out, UW)
    return gamma.reshape(()) * up + x


if __name__ == "__main__":
    rng = np.random.default_rng(0)
    inputs = {
        "x": rng.standard_normal((B, C, H, W), dtype=np.float32),
        "Wq": (rng.standard_normal((CQ, C)) * 0.05).astype(np.float32),
        "bq": (rng.standard_normal((CQ,)) * 0.05).astype(np.float32),
        "Wk": (rng.standard_normal((CQ, C)) * 0.05).astype(np.float32),
        "bk": (rng.standard_normal((CQ,)) * 0.05).astype(np.float32),
        "Wv": (rng.standard_normal((C, C)) * 0.05).astype(np.float32),
        "bv": (rng.standard_normal((C,)) * 0.05).astype(np.float32),
        "gamma": np.full((1,), 0.7, dtype=np.float32),
    }
    y = kernel(**inputs)
    want = _np_reference(**inputs)
    err = np.linalg.norm(y - want) / np.linalg.norm(want)
    print("gamma=0.7 l2 rel err:", err)
    inputs["gamma"] = np.zeros((1,), dtype=np.float32)
    y = kernel(**inputs)
    want = _np_reference(**inputs)
    err = np.linalg.norm(y - want) / np.linalg.norm(want)
    print("gamma=0   l2 rel err:", err)


# revision 24
# speedup vs baseline: 1.1735x; 1.1633x over previous
"""Trainium2 Bass kernel for the AttentionBlock problem (v3).

Math (per batch b, one NeuronCore each):
  x_down = avgpool4x4(x)            # [C, 32, 32] -> xf [C, N], N=1024
  q,k = Wq/Wk @ xf + b              # [8, N]
  v = Wv @ xf + bv                  # [C, N]
  attn = softmax_n(q^T k)           # [N, N]
  out[c,m] = sum_n v[c,n] attn[m,n]
  y = gamma * upsample_bilinear(out) + x

v3 design (cost-model driven):
  - DMA is the serial floor: 8.4MB in + 8.4MB out bf16 ~= 46.6us. Everything
    else is hidden under it: pooling + projections + m-half-0 logits during
    the input stream, a ~3us bridge, then a DMA-paced output stream.
  - All attention-branch matmuls run in fp8e4m3 with DoubleRow perf mode
    (0.5 cyc/col, 256-deep contraction): v/q/k projections contract the two
    c-halves at once; attn@V contracts n-chunk pairs; the fused
    upsample+gamma matmul contracts two m-chunks at once. The residual path
    stays bf16, so with gamma==0 the output is exactly the bf16 roundtrip
    of x.
  - Bilinear row weights repeat every 4 slices, so 6 static "universal
    slabs" [128m, 2kt, 4r x 128w] (4 interior phases + 2 edges) replace
    per-slice weight generation. They are built on-device from a tiny
    host table A (x gamma) and the W-resize matrix B: slab = A*B, fp8 out.
    A dummy zero chunk on each end of o_sb keeps every slice a contiguous
    chunk pair.
  - Output phase per si-pair (1024 cols, 2 c-halves): 2 fp8 up-matmuls per
    half; ch0 drains on DVE (scalar_tensor_tensor: psum + gamma*bv + x),
    ch1 folds x into PSUM via an identity matmul and drains on ACT
    (Identity activation with per-partition gamma*bv bias). Softmax
    denominators ride as a ones-column in Vt; reciprocals are batched in
    mk pairs; rec-scale happens in the PSUM->SBUF fp8 copy of O.
  - q/k biases are applied by the ACT copy (per-partition bias + 1/8 scale
    undoing the fp8 weight scaling); bv is applied as the per-partition
    drain bias (bilinear rows sum to 1); 1/16 pooling mean lives in the
    xf->fp8 convert; Wv's 4x fp8 scaling is undone inside the slab table.
"""

import numpy as np

B, C, H, W = 8, 256, 128, 128
HD, WD = 32, 32
N = HD * WD  # 1024
CQ = 8
NCORES = 8

_CACHE = {}


def _resize_matrix(dst: int, src: int) -> np.ndarray:
    """Bilinear (half-pixel, edge-renormalized) resize matrix, matches
    jax.image.resize(method='linear') for upsampling."""
    scale = dst / src
    pos = (np.arange(dst, dtype=np.float64) + 0.5) / scale - 0.5
    j = np.arange(src, dtype=np.float64)
    w = np.maximum(0.0, 1.0 - np.abs(pos[:, None] - j[None, :]))
    w = w / w.sum(axis=1, keepdims=True)
    return w.astype(np.float32)  # [dst, src]


def _slab_id(si):
    if si == 0:
        return 4
    if si == 31:
        return 5
    return si % 4


def _pair_lo_mk(si):
    a = si // 4
    if si == 0:
        return -1  # (dummy, mk0)
    if si % 4 == 0:
        return a - 1
    if si >= 29:
        return 7
    return a


def _slab_tables():
    """A[p, s, kt*4+r] H-weight table for the 6 universal slabs and the
    per-si (slab id, o_sb dev-chunk lo) map. dev chunk = mk + 1 with dummy
    zero chunks at 0 and 9."""
    UH = _resize_matrix(H, HD)
    reps = {}
    for si in range(32):
        reps.setdefault(_slab_id(si), si)
    A = np.zeros((128, 6, 8), np.float32)
    for s, si in reps.items():
        lo = _pair_lo_mk(si)
        for kt in range(2):
            mk = lo + kt
            if not (0 <= mk <= 7):
                continue
            for sub in range(4):
                hb = mk * 4 + sub
                if abs(hb - si) <= 1 and 0 <= hb < 32:
                    for r in range(4):
                        A[sub * 32:(sub + 1) * 32, s, kt * 4 + r] = \
                            UH[4 * si + r, hb]
    # universality check: the rep-si table must reproduce every si exactly
    for si in range(32):
        s, lo = _slab_id(si), _pair_lo_mk(si)
        for kt in range(2):
            mk = lo + kt
            for sub in range(4):
                hb = mk * 4 + sub
                want = (UH[4 * si:4 * si + 4, hb] if 0 <= mk <= 7 and hb < 32
                        else np.zeros(4, np.float32))
                got = A[sub * 32, s, kt * 4:kt * 4 + 4]
                assert np.allclose(got, want), (si, kt, sub, got, want)
    perm = {4: 0, 1: 1, 2: 2, 3: 3, 0: 4, 5: 5}
    simap = [(perm[_slab_id(si)], _pair_lo_mk(si) + 1) for si in range(32)]
    return A, simap


_A_TABLE, _SIMAP = _slab_tables()


def _build_bass():
    import concourse.bass as bass
    import concourse.tile as tile
    from concourse import bacc, mybir

    f32 = mybir.dt.float32
    bf16 = mybir.dt.bfloat16
    fp8 = mybir.dt.float8e4
    AF = mybir.ActivationFunctionType
    AL = mybir.AluOpType
    DR = mybir.MatmulPerfMode.DoubleRow

    nc = bacc.Bacc("TRN2", target_bir_lowering=False, debug=False)

    x_d = nc.dram_tensor("x", [C, H * W], bf16, kind="ExternalInput")
    wqk_d = nc.dram_tensor("wqk8", [128, 2 * 48], fp8, kind="ExternalInput")
    bqk_d = nc.dram_tensor("bqk", [40, 1], f32, kind="ExternalInput")
    wv_d = nc.dram_tensor("wv8", [128, 2 * C], fp8, kind="ExternalInput")
    gbv_d = nc.dram_tensor("gbv", [128, 2], f32, kind="ExternalInput")
    slabA_d = nc.dram_tensor("slabA", [128, 3 * 1024], fp8,
                             kind="ExternalInput")
    slabB_d = nc.dram_tensor("slabB", [128, 3 * 1024], fp8,
                             kind="ExternalInput")
    eye_d = nc.dram_tensor("eye", [128, 128], bf16, kind="ExternalInput")
    y_d = nc.dram_tensor("y", [C, H * W], bf16, kind="ExternalOutput")

    with tile.TileContext(nc) as tc:
        with (
            tc.tile_pool(name="xbig", bufs=1) as xbig,
            tc.tile_pool(name="persist", bufs=1) as persist,
            tc.tile_pool(name="ptmp", bufs=2) as ptmp,
            tc.tile_pool(name="ps_lt", bufs=1, space="PSUM") as ps_lt,
            tc.tile_pool(name="ps_o", bufs=1, space="PSUM") as ps_o,
        ):
            x0 = xbig.tile([128, H * W], bf16)
            x1 = xbig.tile([128, H * W], bf16)
            xt = [x0, x1]

            et_sb = persist.tile([128, 8, N], fp8)       # Et[nk][n_l, m]
            vt_sb = persist.tile([128, 8, 272], fp8)     # Vt[nk][n_l, c|1|pad]
            o_sb = persist.tile([128, 10, C], fp8)       # O[dev mk][m_l, c]
            slab_sb = persist.tile([128, 6, 2, 4, W], fp8)
            xf8_sb = persist.tile([128, 2, N], fp8)
            q_sb = persist.tile([CQ, N], bf16)
            k_sb = persist.tile([CQ, N], bf16)
            rec_sb = persist.tile([128, 8], f32)
            wqk_sb = persist.tile([128, 2, 48], fp8)
            bqk_sb = persist.tile([40, 1], f32)
            wv_sb = persist.tile([128, 2, C], fp8)
            gbv_sb = persist.tile([128, 2], f32)
            eye_sb = persist.tile([128, 128], bf16)

            ps_qkvt_cm = tc.tile_pool(name="ps_qkvt", bufs=2, space="PSUM")
            ps_qkvt = ps_qkvt_cm.__enter__()

            # ---------- helpers ----------
            def pool_strip(t, st, off, ln):
                # 4x4 sum entirely on DVE; the final add writes fp8 directly
                # (scales undone in copy scales / the 64.0 ones column).
                nh = ln // 512
                strip = xt[t][:, bass.ds(st * 2048 + off, ln)]
                v1 = strip.rearrange("p (h two w) -> p h two w", two=2, w=128)
                t1 = ptmp.tile([128, nh * 2, 128], bf16,
                               tag=f"t1_{t}_{off}_{ln}",
                               name=f"t1_{t}_{off}_{ln}")
                nc.vector.tensor_add(t1[:], v1[:, :, 0, :], v1[:, :, 1, :])
                v2 = t1[:].rearrange("p (h two) w -> p h two w", two=2)
                t2 = ptmp.tile([128, nh, 128], bf16,
                               tag=f"t2_{t}_{off}_{ln}",
                               name=f"t2_{t}_{off}_{ln}")
                nc.vector.tensor_add(t2[:], v2[:, :, 0, :], v2[:, :, 1, :])
                v3 = t2[:].rearrange("p hb (wp two) -> p hb wp two", two=2)
                t3 = ptmp.tile([128, nh, 64], bf16,
                               tag=f"t3_{t}_{off}_{ln}",
                               name=f"t3_{t}_{off}_{ln}")
                nc.gpsimd.tensor_add(t3[:], v3[:, :, :, 0], v3[:, :, :, 1])
                v4 = t3[:].rearrange("p hb (wb two) -> p hb wb two", two=2)
                xfs = xf8_sb[:, t, bass.ds(st * 128 + off // 16, ln // 16)
                             ].rearrange("p (hb wb) -> p hb wb", hb=nh)
                nc.gpsimd.tensor_add(xfs, v4[:, :, :, 0], v4[:, :, :, 1])

            def vt_mm(nk):
                nsl = bass.ds(nk * 128, 128)
                vt_ps = ps_qkvt.tile([128, C], f32, tag="vt", name="vt_ps")
                nc.tensor.matmul(vt_ps[:], xf8_sb[:, :, nsl], wv_sb[:],
                                 start=True, stop=True, perf_mode=DR)
                return vt_ps

            def vt_copy(nk, vt_ps):
                if nk % 2 == 0:
                    nc.vector.tensor_copy(vt_sb[:, nk, 0:C], vt_ps[:])
                else:
                    nc.scalar.copy(vt_sb[:, nk, 0:C], vt_ps[:])

            def qk_mm(st):
                nsl = bass.ds(st * 128, 128)
                qk_ps = ps_qkvt.tile([48, 128], f32, tag="qk", name="qk_ps")
                nc.tensor.matmul(qk_ps[:], wqk_sb[:], xf8_sb[:, :, nsl],
                                 start=True, stop=True, perf_mode=DR)
                return qk_ps

            def q_copy(st, qk_ps):
                nsl = bass.ds(st * 128, 128)
                nc.scalar.activation(q_sb[:, nsl], qk_ps[0:CQ, :],
                                     func=AF.Identity, bias=bqk_sb[0:CQ, :],
                                     scale=1.0 / 128.0)

            def k_copy(st, qk_ps):
                nsl = bass.ds(st * 128, 128)
                nc.scalar.activation(k_sb[:, nsl], qk_ps[32:40, :],
                                     func=AF.Identity, bias=bqk_sb[32:40, :],
                                     scale=1.0 / 128.0)

            def do_logits(nk, m0, mlen, lt=None):
                nsl = bass.ds(nk * 128, 128)
                if lt is None:
                    lt = ps_lt.tile([128, 512], f32, tag="lt", name="lt_ps")
                nc.tensor.matmul(lt[:, 0:mlen], k_sb[:, nsl],
                                 q_sb[:, bass.ds(m0, mlen)],
                                 start=True, stop=True)
                nc.scalar.activation(et_sb[:, nk, bass.ds(m0, mlen)],
                                     lt[:, 0:mlen], func=AF.Exp)
                return lt

            o_ps_holder = {}

            def attnv_mm(oj, mk, pks, start):
                o_ps = o_ps_holder["t"]
                msl = bass.ds(mk * 128, 128)
                for i, pk in enumerate(pks):
                    nc.tensor.matmul(o_ps[:, oj, 0:C + 1],
                                     et_sb[:, pk:pk + 2, msl],
                                     vt_sb[:, pk:pk + 2, 0:C + 1],
                                     start=(start and i == 0),
                                     stop=(pk == 6), perf_mode=DR)

            def attnv_fin(mk0):
                # mk0, mk0+1 live in o_ps halves 0/1; batch the recip.
                o_ps = o_ps_holder["t"]
                nc.vector.reciprocal(rec_sb[:, mk0:mk0 + 2], o_ps[:, :, C])
                nc.vector.tensor_scalar(out=o_sb[:, mk0 + 1, :],
                                        in0=o_ps[:, 0, 0:C],
                                        scalar1=rec_sb[:, mk0:mk0 + 1],
                                        scalar2=None, op0=AL.mult)
                nc.scalar.mul(o_sb[:, mk0 + 2, :], o_ps[:, 1, 0:C],
                              mul=rec_sb[:, mk0 + 1:mk0 + 2])

            # ---------- input + bridge: 2-stage software pipeline ----------
            # stage st: pool(st) [DMA-gated only]; vt/qk matmuls of st-1;
            # logits mms + exps of st-2; copies of st-1. Stages 8/9 drain
            # the pipeline (bridge).
            def exp_items(j):
                items = []
                if j >= 4:
                    items.append(([j], 0, 512))
                    items.append(([j], 512, (j + 1) * 128 - 512))
                else:
                    items.append(([j], 0, (j + 1) * 128))
                if j >= 1:
                    items.append((list(range(min(j, 4))), j * 128, 128))
                if j >= 5:
                    items.append((list(range(4, j)), j * 128, 128))
                return items

            stash = {}
            for st in range(9):
                # a) DMAs
                if st == 7:
                    for sl0, ln in ((0, 1024), (1024, 1024)):
                        for t in range(2):
                            sl = bass.ds(st * 2048 + sl0, ln)
                            nc.sync.dma_start(
                                out=xt[t][:, sl],
                                in_=x_d[t * 128:(t + 1) * 128, sl])
                    nc.sync.dma_start(
                        out=slab_sb[:, 0:3],
                        in_=slabA_d[:].rearrange(
                            "p (s kt r w) -> p s kt r w", s=3, kt=2, r=4))
                    nc.sync.dma_start(
                        out=slab_sb[:, 3:6],
                        in_=slabB_d[:].rearrange(
                            "p (s kt r w) -> p s kt r w", s=3, kt=2, r=4))
                elif st < 7:
                    for t in range(2):
                        sl = bass.ds(st * 2048, 2048)
                        nc.sync.dma_start(out=xt[t][:, sl],
                                          in_=x_d[t * 128:(t + 1) * 128, sl])
                if st == 0:
                    nc.sync.dma_start(
                        out=wqk_sb[:],
                        in_=wqk_d[:].rearrange("p (k m) -> p k m", k=2))
                    nc.sync.dma_start(out=bqk_sb[:], in_=bqk_d[:])
                    nc.sync.dma_start(
                        out=wv_sb[:],
                        in_=wv_d[:].rearrange("p (k m) -> p k m", k=2))
                    nc.gpsimd.memset(vt_sb[:, :, C:C + 1], 64.0)
                    nc.gpsimd.memset(o_sb[:, 0, :], 0.0)
                    nc.gpsimd.memset(o_sb[:, 9, :], 0.0)
                if st == 1:
                    nc.sync.dma_start(out=gbv_sb[:], in_=gbv_d[:])
                    nc.sync.dma_start(out=eye_sb[:], in_=eye_d[:])

                # b) PE projections of st-1 (xf8(st-1) just landed)
                if st >= 1:
                    stash[("vt", st - 1)] = vt_mm(st - 1)
                    stash[("qk", st - 1)] = qk_mm(st - 1)

                # c) pooling (DMA-gated only)
                if st < 7:
                    pool_strip(0, st, 0, 2048)
                    pool_strip(1, st, 0, 2048)
                elif st == 7:
                    pool_strip(0, st, 0, 1024)
                    pool_strip(1, st, 0, 1024)
                    pool_strip(0, st, 1024, 1024)
                    pool_strip(1, st, 1024, 1024)

                # d/e) copies of st-1 (after pooling on DVE; ACT head for k)
                if st >= 1:
                    vt_copy(st - 1, stash.pop(("vt", st - 1)))
                    qk_ps = stash.pop(("qk", st - 1))
                    q_copy(st - 1, qk_ps)
                    k_copy(st - 1, qk_ps)

                # f/g) logits + exps of st-1 (j=7 row goes to bridge/sides)
                if st >= 1:
                    j = st - 1
                    items = exp_items(j) if j < 7 else [([7], 0, 512)]
                    for nks, m0, mlen in items:
                        lt = logits_mm(nks, m0, mlen)
                        logits_exp(nks, m0, mlen, lt)

                # h) attnv prefill for mk0/mk1
                if st == 3:
                    o_ps_holder["t"] = ps_o.tile([128, 2, 512], f32, tag="o",
                                                 name="o_ps")
                    attnv_mm(0, 0, [0], True)
                    attnv_mm(1, 1, [0], True)
                if st == 5:
                    attnv_mm(0, 0, [2], False)
                    attnv_mm(1, 1, [2], False)
                if st == 7:
                    attnv_mm(0, 0, [4], False)
                    attnv_mm(1, 1, [4], False)

            # ---------- bridge tail ----------
            attnv_mm(0, 0, [6], False)
            attnv_mm(1, 1, [6], False)
            attnv_fin(0)

            ps_qkvt_cm.__exit__(None, None, None)
            ps_y_cm = tc.tile_pool(name="ps_y", bufs=4, space="PSUM")
            ps_y = ps_y_cm.__enter__()

            def up_pair(p, ch):
                y_ps = ps_y.tile([128, 1024], f32, tag="y", name="y_ps")
                for j in (0, 1):
                    si = 2 * p + j
                    s, dlo = _SIMAP[si]
                    reg = y_ps[:, j * 512:(j + 1) * 512]
                    nc.tensor.matmul(
                        reg, o_sb[:, dlo:dlo + 2, ch * 128:(ch + 1) * 128],
                        slab_sb[:, s].rearrange("p kt r w -> p kt (r w)"),
                        start=True, stop=(ch == 0), perf_mode=DR)
                    if ch == 1:
                        nc.tensor.matmul(
                            reg, eye_sb[:],
                            xt[1][:, bass.ds(si * 512, 512)],
                            start=False, stop=True)
                psl = bass.ds(p * 1024, 1024)
                if ch == 0:
                    nc.vector.scalar_tensor_tensor(
                        out=xt[0][:, psl], in0=y_ps[:],
                        scalar=gbv_sb[:, 0:1], in1=xt[0][:, psl],
                        op0=AL.add, op1=AL.add)
                else:
                    nc.scalar.activation(xt[1][:, psl], y_ps[:],
                                         func=AF.Identity,
                                         bias=gbv_sb[:, 1:2], scale=1.0)
                nc.sync.dma_start(out=y_d[ch * 128:(ch + 1) * 128, psl],
                                  in_=xt[ch][:, psl])

            def side(p):
                if p == 0:
                    do_logits(7, 512, 512)
                elif p == 1:
                    do_logits(0, 896, 128)
                    do_logits(1, 896, 128)
                    do_logits(2, 896, 128)
                elif p == 2:
                    attnv_mm(0, 2, [0, 2, 4, 6], True)
                    attnv_mm(1, 3, [0, 2, 4, 6], True)
                    attnv_fin(2)
                elif p == 3:
                    do_logits(3, 896, 128)
                    do_logits(4, 896, 128)
                    do_logits(5, 896, 128)
                elif p == 4:
                    do_logits(6, 896, 128)
                    attnv_mm(0, 4, [0, 2, 4, 6], True)
                    attnv_mm(1, 5, [0, 2, 4, 6], True)
                    attnv_fin(4)
                elif p == 6:
                    attnv_mm(0, 6, [0, 2, 4, 6], True)
                    attnv_mm(1, 7, [0, 2, 4, 6], True)
                    attnv_fin(6)

            for p in range(16):
                up_pair(p, 0)
                up_pair(p, 1)
                side(p)

            ps_y_cm.__exit__(None, None, None)
    nc.compile()
    return nc


def _get_nc():
    if "nc" not in _CACHE:
        _CACHE["nc"] = _build_bass()
    return _CACHE["nc"]


def kernel(x, Wq, bq, Wk, bk, Wv, bv, gamma):
    from concourse.bass_utils import run_bass_kernel_spmd
    import ml_dtypes

    bfd = ml_dtypes.bfloat16
    f8d = ml_dtypes.float8_e4m3

    x = np.ascontiguousarray(np.asarray(x, dtype=np.float32))
    UW = _resize_matrix(W, WD)  # [128, 32]
    gam_f = float(np.asarray(gamma).reshape(-1)[0])

    p = np.arange(128)
    # B[p, w] = UW[w, p%32]; gamma folded in so slabs are exactly zero
    # when gamma == 0 (the 4x/16x v scalings cancel via the 64.0 ones col).
    Bm = UW[:, p % 32].T * gam_f              # [128, 128]
    # slab[p, s, kt, r, w] = A[p, s, kt*4+r] * Bm[p, w]; DMA slot order
    # [s4, s1, s2 | s3, s0, s5]
    slab = (_A_TABLE[:, :, :, None] * Bm[:, None, None, :]).reshape(
        128, 6, 2, 4, 128)
    slab8 = slab.astype(f8d)
    order = [4, 1, 2, 3, 0, 5]
    slabA = np.ascontiguousarray(
        slab8[:, order[0:3]].reshape(128, 3 * 1024))
    slabB = np.ascontiguousarray(
        slab8[:, order[3:6]].reshape(128, 3 * 1024))
    eye = np.eye(128, dtype=bfd)

    wqk8 = np.zeros((128, 2, 48), dtype=f8d)
    Wqa = np.asarray(Wq, dtype=np.float32)
    Wka = np.asarray(Wk, dtype=np.float32)
    Wva = np.asarray(Wv, dtype=np.float32)
    for h in range(2):
        wqk8[:, h, 0:8] = (8.0 * Wqa[:, h * 128:(h + 1) * 128].T).astype(f8d)
        wqk8[:, h, 32:40] = (8.0 * Wka[:, h * 128:(h + 1) * 128].T
                             ).astype(f8d)
    bqk = np.zeros((40, 1), dtype=np.float32)
    bqk[0:8, 0] = np.asarray(bq, dtype=np.float32)
    bqk[32:40, 0] = np.asarray(bk, dtype=np.float32)
    wv8 = np.zeros((128, 2, C), dtype=f8d)
    for h in range(2):
        wv8[:, h, :] = (4.0 * Wva[:, h * 128:(h + 1) * 128].T).astype(f8d)
    gbv = np.ascontiguousarray(
        np.asarray(bv, dtype=np.float32).reshape(2, 128).T * gam_f)

    nc = _get_nc()
    in_maps = []
    for i in range(NCORES):
        in_maps.append({
            "x": np.ascontiguousarray(x[i].reshape(C, H * W)).astype(bfd),
            "wqk8": np.ascontiguousarray(wqk8.reshape(128, 96)),
            "bqk": bqk.copy(),
            "wv8": np.ascontiguousarray(wv8.reshape(128, 2 * C)),
            "gbv": gbv.copy(),
            "slabA": slabA.copy(),
            "slabB": slabB.copy(),
            "eye": eye.copy(),
        })
    res = run_bass_kernel_spmd(nc, in_maps, core_ids=list(range(NCORES)))
    y = np.stack([np.asarray(r["y"], dtype=np.float32).reshape(C, H, W)
                  for r in res.results])
    return y


def _np_reference(x, Wq, bq, Wk, bk, Wv, bv, gamma):
    b, c, h, w = x.shape
    hd, wd = h // 4, w // 4
    xd = x.reshape(b, c, hd, 4, wd, 4).mean(axis=(3, 5))
    xf = xd.reshape(b, c, hd * wd)
    q = np.einsum('oc,bcn->bon', Wq, xf) + bq[None, :, None]
    k = np.einsum('oc,bcn->bon', Wk, xf) + bk[None, :, None]
    v = np.einsum('oc,bcn->bon', Wv, xf) + bv[None, :, None]
    lg = np.einsum('bcm,bcn->bmn', q, k)
    lg = np.exp(lg - lg.max(axis=-1, keepdims=True))
    attn = lg / lg.sum(axis=-1, keepdims=True)
    out = np.einsum('bcn,bmn->bcm', v, attn).reshape(b, c, hd, wd)
    UH = _resize_matrix(h, hd)
    UW = _resize_matrix(w, wd)
    up = np.einsum('hj,bcjk,wk->bchw', UH, out, UW)
    return gamma.reshape(()) * up + x


if __name__ == "__main__":
    rng = np.random.default_rng(0)
    inputs = {
        "x": rng.standard_normal((B, C, H, W), dtype=np.float32),
        "Wq": (rng.standard_normal((CQ, C)) * 0.05).astype(np.float32),
        "bq": (rng.standard_normal((CQ,)) * 0.05).astype(np.float32),
        "Wk": (rng.standard_normal((CQ, C)) * 0.05).astype(np.float32),
        "bk": (rng.standard_normal((CQ,)) * 0.05).astype(np.float32),
        "Wv": (rng.standard_normal((C, C)) * 0.05).astype(np.float32),
        "bv": (rng.standard_normal((C,)) * 0.05).astype(np.float32),
        "gamma": np.full((1,), 0.7, dtype=np.float32),
    }
    y = kernel(**inputs)
    want = _np_reference(**inputs)
    err = np.linalg.norm(y - want) / np.linalg.norm(want)
    print("gamma=0.7 l2 rel err:", err)
    inputs["gamma"] = np.zeros((1,), dtype=np.float32)
    y = kernel(**inputs)
    want = _np_reference(**inputs)
    err = np.linalg.norm(y - want) / np.linalg.norm(want)
    print("gamma=0   l2 rel err:", err)
